# revision 21
# baseline (speedup 1.0000x reference)
"""Multi-head attention (B=4, N=2048, D=1024, H=16) on 8 Trainium2 NeuronCores.

Sharding: core c = 2*b + hg handles batch b and head-group hg (8 of 16 heads).
Host pre-transposes x and mask per batch, slices Wq/Wk/Wv columns and Wo rows
per head group, folds the v-bias into the output bias (softmax weights sum to
1, so ctx = p@(v+bv) = p@v + bv), and sums the two partial outputs per batch
(+ bo') at the end.

Per-core kernel, per-kc (128 keys) software pipeline:
  scores (PE, 2 heads row-packed) -> mask-mult (DVE, psum->sbuf f16, frees
  psum after ONE pass) -> exp (ACT, 4-kc batches) -> ctx (PE, per-head
  [65,512] psum banks whose 65th weight column of ones accumulates the
  softmax denominator for free).  Per-pair normalize: ln on the two psum
  denominator rows (ACT, psum-direct), exp(-x) -> 1/d, gpsimd
  partition_broadcast to 64 rows, two DVE mults, and a partition-shift DMA
  that moves the hp1 half-context up to partitions 64-127.
  qk projections for pairs 1-3 are emitted inside the first q-chunk's rows
  so they overlap the DVE/ACT-paced attention pipeline; the previous
  q-chunk's output projection is interleaved likewise and DMAs straight
  from PSUM to DRAM.
"""
import os
from contextlib import ExitStack

import numpy as np

from concourse import bacc, mybir, tile
from concourse import bass_utils

P = 128
NSEQ = 2048          # sequence length
DMODEL = 1024        # model dim
HD = 512             # per-core head dim total (8 heads x 64)
NPAIR = 4            # head pairs per core
DH = 64              # head depth
DC = DMODEL // P     # 8 d_model chunks
NQC = 4              # q chunks of 512
NKC = 16             # k chunks of 128
F32 = mybir.dt.float32
F16 = mybir.dt.float16
AF = mybir.ActivationFunctionType
OP = mybir.AluOpType

_CACHE: dict = {}
LAST_RESULTS = None

# crash-bisect flags (default: all features on)
_NO_QKINT = bool(os.environ.get("KB_NO_QKINT"))
_PSUM_Y = bool(os.environ.get("KB_PSUM_Y"))  # DMA y from psum (bass forbids)


def _patch_act_tables():
    """Force every activation onto the one table set containing
    exp+ln+copy+identity, so the kernel performs a single ACT_TABLE_LOAD
    instead of thrashing between per-function sets (1.3us each)."""
    import functools
    from concourse import bacc as _bacc
    from concourse import hw_specs as _hw
    if getattr(_bacc, "_act_tables_patched", False):
        return
    orig = _hw.get_activation_tables

    @functools.cache
    def patched(arch):
        tabs = dict(orig(arch))
        full = "natural_log_exp_and_others"
        keep = tabs[full]
        strip = {f for f in keep}
        out = {}
        for name, funcs in tabs.items():
            out[name] = funcs if name == full else (funcs - strip)
        return out

    _bacc.get_activation_tables = patched
    _bacc._act_tables_patched = True


def _build():
    _patch_act_tables()
    nc = bacc.Bacc("TRN2", target_bir_lowering=False, debug=False,
                   enable_asserts=False, num_devices=8)

    xT = nc.dram_tensor("xT", [DMODEL, NSEQ], F16, kind="ExternalInput").ap()
    maskT = nc.dram_tensor("maskT", [NSEQ, NSEQ], F16, kind="ExternalInput").ap()
    wq_d = nc.dram_tensor("wq", [DMODEL, HD], F16, kind="ExternalInput").ap()
    wk_d = nc.dram_tensor("wk", [DMODEL, HD], F16, kind="ExternalInput").ap()
    wv_d = nc.dram_tensor("wv", [DMODEL, HD], F16, kind="ExternalInput").ap()
    wo_d = nc.dram_tensor("wo", [P, NPAIR, DMODEL], F16, kind="ExternalInput").ap()
    bq_d = nc.dram_tensor("bq2", [P, NPAIR], F32, kind="ExternalInput").ap()
    bk_d = nc.dram_tensor("bk2", [P, NPAIR], F32, kind="ExternalInput").ap()
    ones_d = nc.dram_tensor("ones2", [P, 512], F16, kind="ExternalInput").ap()
    y_d = nc.dram_tensor("y", [NSEQ, DMODEL], F32, kind="ExternalOutput").ap()

    xT_r = xT.rearrange("(dc p) n -> p dc n", p=P)        # [128, 8, 2048]
    maskT_r = maskT.rearrange("(kc p) q -> p kc q", p=P)  # [128, 16, 2048]

    with tile.TileContext(nc) as tc, ExitStack() as ctx:
        persist = ctx.enter_context(tc.tile_pool(name="persist", bufs=1))
        x = persist.tile([P, DC, NSEQ], F16)     # resident input (transposed)
        qT = persist.tile([P, NPAIR, NSEQ], F16)  # [hd%128, pair, seq]
        kT = persist.tile([P, NPAIR, NSEQ], F16)
        # v with a 65th all-ones weight column per (kc, pair, hp): the ctx
        # matmul then accumulates the softmax denominator into psum row 64.
        # Flat with 64 pad cols so every slot supports a 128-col window.
        NV = NKC * NPAIR * 2 * 65
        v65f = persist.tile([P, NV + 64], F16)
        v65 = v65f[:, 0:NV].rearrange("p (kc pr hp c) -> p kc pr hp c",
                                      kc=NKC, pr=NPAIR, hp=2)
        wq = persist.tile([P, DC, HD], F16)
        wk = persist.tile([P, DC, HD], F16)
        wv = persist.tile([P, DC, HD], F16)
        wo = persist.tile([P, NPAIR, DMODEL], F16)
        ones = persist.tile([P, 512], F16)
        ones65 = persist.tile([65, 64], F16)
        bqs = persist.tile([P, NPAIR], F32)
        bks = persist.tile([P, NPAIR], F32)

        nc.sync.dma_start(out=ones, in_=ones_d)
        nc.sync.dma_start(out=bqs, in_=bq_d)
        nc.sync.dma_start(out=bks, in_=bk_d)
        nc.sync.dma_start(out=wv, in_=wv_d.rearrange("(dc p) m -> p dc m", p=P))
        # stage x per seq-chunk so v-projection can start early
        for dc in range(DC):
            nc.sync.dma_start(out=x[:, dc, 0:512], in_=xT_r[:, dc, 0:512])
        nc.sync.dma_start(out=wq, in_=wq_d.rearrange("(dc p) m -> p dc m", p=P))
        nc.sync.dma_start(out=wk, in_=wk_d.rearrange("(dc p) m -> p dc m", p=P))
        for n in range(1, NQC):
            for dc in range(DC):
                nc.sync.dma_start(out=x[:, dc, n * 512:(n + 1) * 512],
                                  in_=xT_r[:, dc, n * 512:(n + 1) * 512])
        nc.sync.dma_start(out=wo, in_=wo_d)

        ssp = ctx.enter_context(tc.tile_pool(name="ssp", bufs=2, space="PSUM"))
        pcp = ctx.enter_context(tc.tile_pool(name="pcp", bufs=4, space="PSUM"))
        mkpool = ctx.enter_context(tc.tile_pool(name="mk", bufs=3))
        exmpool = ctx.enter_context(tc.tile_pool(name="exm", bufs=2))
        expool = ctx.enter_context(tc.tile_pool(name="ex", bufs=2))
        cxpool = ctx.enter_context(tc.tile_pool(name="cx", bufs=4))
        clpool = ctx.enter_context(tc.tile_pool(name="cl", bufs=2))
        trpool = ctx.enter_context(tc.tile_pool(name="tr", bufs=2))
        ypool = ctx.enter_context(tc.tile_pool(name="yo", bufs=2))
        rbpool = ctx.enter_context(tc.tile_pool(name="rb", bufs=2))

        # the 65th (ones) weight column of every v tile, set once
        nc.vector.memset(v65[:, :, :, :, 64:65], 1.0)
        nc.vector.memset(v65f[:, NV:], 0.0)
        # broadcast-matmul weights: ones row at partition 64 (aligned with
        # the denominator lane the normalize chain lives on)
        nc.vector.memset(ones65[64:65, :], 1.0)

        # ---- PE warmup (HAM) ----
        wt = ssp.tile([P, 2, 512], F32, tag="ss", name="warm")
        for i in range(16):
            nc.tensor.matmul(wt[:, i % 2, :], lhsT=ones[:, 0:P], rhs=ones,
                             start=(i < 2), stop=(i >= 14))

        # ---- v projection into v65 (columns 0..63 per (kc, pair, hp)) ----
        for n in range(NQC):
            for s2 in range(2):
                psv = ssp.tile([P, 2, 512], F32, tag="ss", name="psv")
                for j in range(2):
                    s = n * 4 + s2 * 2 + j
                    for dc in range(DC):
                        nc.tensor.matmul(
                            psv[:, j, :],
                            lhsT=x[:, dc, s * 128:(s + 1) * 128],
                            rhs=wv[:, dc, :],
                            start=(dc == 0), stop=(dc == DC - 1))
                for j in range(2):
                    s = n * 4 + s2 * 2 + j
                    # psv free layout: 512 = (pair, hp, 64); scatter into the
                    # 65-strided v65 slots (v-bias folded into bo on host)
                    nc.scalar.activation(
                        out=v65[:, s, :, :, 0:64],
                        in_=psv[:, j, :].rearrange("p (pr hp d) -> p pr hp d",
                                                   pr=NPAIR, hp=2),
                        func=AF.Copy, scale=1.0)

        # ---- q/k projection piece: one (q-or-k, seq-chunk) column ----
        def qk_piece(p, idx):
            w_sb, b_sb, dst = ((wq, bqs, qT), (wk, bks, kT))[idx // 4]
            n = idx % 4
            ps = ssp.tile([P, 2, 512], F32, tag="ss", name="qk")
            for dc in range(DC):
                nc.tensor.matmul(
                    ps[:, 0, :],
                    lhsT=w_sb[:, dc, p * 128:(p + 1) * 128],
                    rhs=x[:, dc, n * 512:(n + 1) * 512],
                    start=(dc == 0), stop=(dc == DC - 1))
            nc.scalar.activation(
                out=dst[:, p, n * 512:(n + 1) * 512], in_=ps[:, 0, :],
                func=AF.Identity, bias=b_sb[:, p:p + 1], scale=1.0)

        def qk_proj(p):
            for idx in range(8):
                qk_piece(p, idx)

        qk_proj(0)
        if _NO_QKINT:
            for p in range(1, NPAIR):
                qk_proj(p)

        # ---- output projection for one q-chunk quarter (qs) ----
        def outproj(qc, cps, qs):
            q0 = qc * 512
            py = ssp.tile([P, 2, 512], F32, tag="ss", name="py")
            for dm in range(2):
                for c in range(NPAIR):
                    nc.tensor.matmul(
                        py[:, dm, :],
                        lhsT=cps[c // 2][:, c % 2, qs * 128:(qs + 1) * 128],
                        rhs=wo[:, c, dm * 512:(dm + 1) * 512],
                        start=(c == 0), stop=(c == NPAIR - 1))
            ydst = y_d[q0 + qs * 128:q0 + (qs + 1) * 128, :].rearrange(
                "q (dm n) -> q dm n", n=512)
            if _PSUM_Y:
                nc.sync.dma_start(out=ydst, in_=py)
            else:
                ysb = ypool.tile([P, 2, 512], F32, tag="y")
                nc.scalar.activation(out=ysb, in_=py, func=AF.Copy, scale=1.0)
                nc.sync.dma_start(out=ydst, in_=ysb)

        # ---- attention ----
        # The PE/DVE/ACT queues execute in order, so anything that waits on
        # a cross-engine chain is EMITTED late enough that its inputs are
        # already computed when it reaches the queue head:
        #   - a batch's ctx matmuls are emitted at the NEXT exp event
        #   - a pair's normalize is emitted in 3 stages at kc 5/6/7 of the
        #     NEXT pair
        prev = None       # (qc, cps) awaiting deferred output projection
        pend_ctx = None   # (pcs, p, ex, kc0): ctx batch awaiting emission
        pend_norm = None  # dict: normalize job carried across stages

        def emit_ctx(job):
            pcs_, p_, ex_, k0 = job
            for j in range(4):
                kcj = k0 + j
                for hp in range(2):
                    # 65-col weights stream at half rate (the 65th col
                    # crosses the 64-col PE tile boundary); read a 128-col
                    # window instead — cols 65-127 are the next slot's data
                    # and only pollute unread psum rows 65-127
                    base = ((kcj * NPAIR + p_) * 2 + hp) * 65
                    nc.tensor.matmul(
                        pcs_[hp][:, :],
                        lhsT=v65f[:, base:base + 128],
                        rhs=ex_[:, j, hp, :],
                        start=(kcj == 0), stop=(kcj == NKC - 1))

        def norm_stage_a(job):
            # ln of the two psum denominator rows (lane 64)
            tAB = trpool.tile([65, 2, 512], F32, tag="tr")
            for hp in range(2):
                nc.scalar.activation(out=tAB[64:65, hp, :],
                                     in_=job["pcs"][hp][64:65, :], func=AF.Ln)
            job["tAB"] = tAB

        def norm_stage_b(job):
            # 1/d = exp(-ln d), then broadcast to 64 psum rows via
            # 1-contraction matmuls (ones row at partition 64)
            rAB = trpool.tile([65, 2, 512], F16, tag="tr")
            nc.scalar.activation(out=rAB[64:65, :, :],
                                 in_=job["tAB"][64:65, :, :], func=AF.Exp,
                                 scale=-1.0)
            r2ps = ssp.tile([P, 2, 512], F32, tag="ss", name="r2ps")
            for hp in range(2):
                nc.tensor.matmul(r2ps[0:64, hp, :], lhsT=ones65[64:65, :],
                                 rhs=rAB[64:65, hp, :], start=True, stop=True)
            job["r2ps"] = r2ps

        def norm_stage_c(job):
            # 1/d to sbuf (single psum input for the DVE mults), normalize
            # both half-contexts, partition-shift hp1 up to rows 64-127
            cps_, p_ = job["cps"], job["p"]
            pcA_, pcB_ = job["pcs"]
            r2b = rbpool.tile([64, 2, 512], F16, tag="rb")
            nc.scalar.activation(out=r2b, in_=job["r2ps"][0:64, :, :],
                                 func=AF.Copy, scale=1.0)
            if p_ % 2 == 0:
                cp2 = cxpool.tile([P, 2, 512], F16, tag="cx")
                cps_.append(cp2)
            else:
                cp2 = cps_[-1]
            nc.vector.tensor_tensor(cp2[0:64, p_ % 2, :], pcA_[0:64, :],
                                    r2b[:, 0, :], OP.mult)
            cpl = clpool.tile([64, 512], F16, tag="cl")
            nc.vector.tensor_tensor(cpl, pcB_[0:64, :], r2b[:, 1, :],
                                    OP.mult)
            nc.sync.dma_start(out=cp2[64:128, p_ % 2, :], in_=cpl)

        def emit_norm_all(job):
            norm_stage_a(job)
            norm_stage_b(job)
            norm_stage_c(job)

        for qc in range(NQC):
            q0 = qc * 512
            mk_tiles = []
            for j in range(2):
                mk = mkpool.tile([P, 8, 512], F16, tag="mk")
                nc.sync.dma_start(
                    out=mk, in_=maskT_r[:, 8 * j:8 * j + 8, q0:q0 + 512])
                mk_tiles.append(mk)
            for p in range(NPAIR):
                # per-head ctx psum bank: rows 0-63 ctx, row 64 denom,
                # rows 65-127 unread garbage from the padded weight window
                pcA = pcp.tile([P, 512], F32, tag="pc", name=f"pcA{qc}_{p}")
                pcB = pcp.tile([P, 512], F32, tag="pc", name=f"pcB{qc}_{p}")
                pcs = (pcA, pcB)
                if p == 0:
                    cps = []
                exm = None
                for kc in range(NKC):
                    ss = ssp.tile([P, 2, 512], F32, tag="ss", name="ss")
                    for hp in range(2):
                        nc.tensor.matmul(
                            ss[:, hp, :],
                            lhsT=kT[64 * hp:64 * hp + 64, p,
                                    kc * 128:(kc + 1) * 128],
                            rhs=qT[64 * hp:64 * hp + 64, p, q0:q0 + 512],
                            start=True, stop=True)
                    if kc % 4 == 0:
                        exm = exmpool.tile([P, 4, 2, 512], F16, tag="exm")
                    mk = mk_tiles[kc // 8]
                    mkb = mk[:, kc % 8, :].unsqueeze(1).broadcast_to(
                        (P, 2, 512))
                    nc.vector.tensor_tensor(exm[:, kc % 4, :, :], ss, mkb,
                                            OP.mult)
                    if kc % 4 == 3:
                        ex = expool.tile([P, 4, 2, 512], F16, tag="ex")
                        nc.scalar.activation(out=ex, in_=exm, func=AF.Exp,
                                             scale=0.125)
                        if pend_ctx is not None:
                            emit_ctx(pend_ctx)
                        pend_ctx = (pcs, p, ex, kc - 3)
                    if pend_norm is not None:
                        if kc == 5:
                            norm_stage_a(pend_norm)
                        elif kc == 6:
                            norm_stage_b(pend_norm)
                        elif kc == 7:
                            norm_stage_c(pend_norm)
                            pend_norm = None
                    # interleave next pair's q/k projection into qc0 rows
                    if (qc == 0 and p < 3 and not _NO_QKINT
                            and kc % 2 == 1):
                        qk_piece(p + 1, (kc - 1) // 2)
                    # interleave previous q-chunk's output projection into
                    # this q-chunk's first row
                    if p == 0 and prev is not None and kc in (9, 11, 13, 15):
                        outproj(prev[0], prev[1], (kc - 9) // 2)
                        if kc == 15:
                            prev = None
                pend_norm = {"cps": cps, "p": p, "pcs": pcs}
            prev = (qc, cps)
        # drain the tail: last ctx batch, last normalize, final outprojs
        emit_ctx(pend_ctx)
        emit_norm_all(pend_norm)
        pend_norm = None
        for qs in range(4):
            outproj(prev[0], prev[1], qs)
    nc.compile()
    return nc


def _get_nc():
    if "nc" not in _CACHE:
        _CACHE["nc"] = _build()
    return _CACHE["nc"]


def kernel(input, mask, Wq, bq, Wk, bk, Wv, bv, Wo, bo):
    x = np.asarray(input, dtype=np.float32)
    m = np.asarray(mask, dtype=np.float32)
    Wq = np.asarray(Wq, dtype=np.float32)
    Wk = np.asarray(Wk, dtype=np.float32)
    Wv = np.asarray(Wv, dtype=np.float32)
    Wo = np.asarray(Wo, dtype=np.float32)
    bq = np.asarray(bq, dtype=np.float32)
    bk = np.asarray(bk, dtype=np.float32)
    bv = np.asarray(bv, dtype=np.float32)
    bo = np.asarray(bo, dtype=np.float32)
    B = x.shape[0]
    assert x.shape == (B, NSEQ, DMODEL) and B == 4

    # softmax weights sum to 1, so the v-bias contributes bv @ Wo to the
    # output exactly; fold it into the output bias
    bo_eff = bo + bv @ Wo

    nc = _get_nc()
    in_maps = []
    for b in range(B):
        xT = np.ascontiguousarray(x[b].T)
        mT = np.ascontiguousarray(m[b].T)
        for hg in range(2):
            sl = slice(hg * HD, (hg + 1) * HD)
            in_maps.append({
                "xT": xT.astype(np.float16),
                "maskT": mT.astype(np.float16),
                "wq": np.ascontiguousarray(Wq[:, sl]).astype(np.float16),
                "wk": np.ascontiguousarray(Wk[:, sl]).astype(np.float16),
                "wv": np.ascontiguousarray(Wv[:, sl]).astype(np.float16),
                "wo": np.ascontiguousarray(
                    Wo[sl].reshape(NPAIR, P, DMODEL).transpose(1, 0, 2)
                ).astype(np.float16),
                "bq2": np.ascontiguousarray(bq[sl].reshape(NPAIR, P).T),
                "bk2": np.ascontiguousarray(bk[sl].reshape(NPAIR, P).T),
                "ones2": np.ones((P, 512), dtype=np.float16),
            })

    res = bass_utils.run_bass_kernel_spmd(nc, in_maps, core_ids=list(range(8)))
    global LAST_RESULTS
    LAST_RESULTS = res

    out = np.empty((B, NSEQ, DMODEL), dtype=np.float32)
    for b in range(B):
        out[b] = res.results[2 * b]["y"] + res.results[2 * b + 1]["y"] + bo_eff
    return out


# revision 22
# speedup vs baseline: 1.4900x; 1.4900x over previous
"""Multi-head attention (B=4, N=2048, D=1024, H=16) on 8 Trainium2 NeuronCores.

Sharding: core c = 2*b + hg handles batch b and head-group hg (8 of 16 heads).
Host pre-transposes x and mask per batch, slices Wq/Wk/Wv columns and Wo rows
per head group, and sums the two partial outputs per batch (+ bo) at the end.

Per-core kernel, per-kc (128 keys) software pipeline:
  scores (PE, 2 heads row-packed) -> mask-mult (DVE, psum->sbuf f16, frees
  psum after ONE pass) -> exp (ACT, 2-kc tiles) -> ctx (PE, 2 heads
  col-tiled into one 128-row psum bank) + denominator (PE, 2 col-packed
  M=1 matmuls).  Per-pair normalize: ln/exp on the two denominator rows, a
  selector-matrix broadcast matmul, one ACT copy, one DVE mult.
  qk projections for pairs 1-3 are emitted inside the first q-chunk's rows
  so they overlap the DVE/ACT-paced attention pipeline.
"""
import os
from contextlib import ExitStack

import numpy as np

from concourse import bacc, mybir, tile
from concourse import bass_utils

P = 128
NSEQ = 2048          # sequence length
DMODEL = 1024        # model dim
HD = 512             # per-core head dim total (8 heads x 64)
NPAIR = 4            # head pairs per core
DH = 64              # head depth
DC = DMODEL // P     # 8 d_model chunks
NQC = 4              # q chunks of 512
NKC = 16             # k chunks of 128
F32 = mybir.dt.float32
F32R = mybir.dt.float32r
F16 = mybir.dt.float16
F8 = mybir.dt.float8e4
DR = mybir.MatmulPerfMode.DoubleRow
AF = mybir.ActivationFunctionType
OP = mybir.AluOpType

_CACHE: dict = {}
LAST_RESULTS = None

# crash-bisect flags (default: all features on)
_NO_DENOM = bool(os.environ.get("KB_NO_DENOM"))
_NO_BCMM2 = bool(os.environ.get("KB_NO_BCMM2"))
_NO_NORM = bool(os.environ.get("KB_NO_NORM"))
_NO_MKBCAST = bool(os.environ.get("KB_NO_MKBCAST"))
_NO_QKINT = bool(os.environ.get("KB_NO_QKINT"))


def _patch_act_tables():
    """Force every activation onto the one table set containing
    exp+ln+copy+identity, so the kernel performs a single ACT_TABLE_LOAD
    instead of thrashing between per-function sets (1.3us each)."""
    import functools
    from concourse import bacc as _bacc
    from concourse import hw_specs as _hw
    if getattr(_bacc, "_act_tables_patched", False):
        return
    orig = _hw.get_activation_tables

    @functools.cache
    def patched(arch):
        tabs = dict(orig(arch))
        full = "natural_log_exp_and_others"
        keep = tabs[full]
        strip = {f for f in keep}
        out = {}
        for name, funcs in tabs.items():
            out[name] = funcs if name == full else (funcs - strip)
        return out

    _bacc.get_activation_tables = patched
    _bacc._act_tables_patched = True


def _patch_ldw_opt():
    """Enable walrus's LDWEIGHTS optimization (dedupes/overlaps weight
    loads); concourse pins it off by default."""
    from concourse import bass_utils as _bu
    if getattr(_bu, "_ldw_opt_patched", False):
        return
    orig = _bu.run_command

    def patched(cmd, *a, **kw):
        cmd = ["--enable-ldw-opt=true" if c == "--enable-ldw-opt=false"
               else c for c in cmd]
        return orig(cmd, *a, **kw)

    _bu.run_command = patched
    _bu._ldw_opt_patched = True


def _build():
    _patch_act_tables()
    if os.environ.get("KB_LDWOPT"):
        _patch_ldw_opt()
    nc = bacc.Bacc("TRN2", target_bir_lowering=False, debug=False,
                   enable_asserts=False, num_devices=8)

    xT = nc.dram_tensor("xT", [DMODEL, NSEQ], F16, kind="ExternalInput").ap()
    maskT = nc.dram_tensor("maskT", [NSEQ, NSEQ], F16, kind="ExternalInput").ap()
    wq_d = nc.dram_tensor("wq", [DMODEL, HD], F16, kind="ExternalInput").ap()
    wk_d = nc.dram_tensor("wk", [DMODEL, HD], F16, kind="ExternalInput").ap()
    wv_d = nc.dram_tensor("wv", [DMODEL, HD], F16, kind="ExternalInput").ap()
    wo_d = nc.dram_tensor("wo", [P, NPAIR, DMODEL], F16, kind="ExternalInput").ap()
    bq_d = nc.dram_tensor("bq2", [P, NPAIR], F32, kind="ExternalInput").ap()
    bk_d = nc.dram_tensor("bk2", [P, NPAIR], F32, kind="ExternalInput").ap()
    bvr_d = nc.dram_tensor("bvr", [P, HD], F32, kind="ExternalInput").ap()
    ones_d = nc.dram_tensor("ones2", [P, 512], F16, kind="ExternalInput").ap()
    ones1_d = nc.dram_tensor("ones1", [P, 1], F16, kind="ExternalInput").ap()
    sel4_d = nc.dram_tensor("sel4", [97, P], F16, kind="ExternalInput").ap()
    y_d = nc.dram_tensor("y", [NSEQ, DMODEL], F32, kind="ExternalOutput").ap()

    xT_r = xT.rearrange("(dc p) n -> p dc n", p=P)        # [128, 8, 2048]
    maskT_r = maskT.rearrange("(kc p) q -> p kc q", p=P)  # [128, 16, 2048]

    with tile.TileContext(nc) as tc, ExitStack() as ctx:
        persist = ctx.enter_context(tc.tile_pool(name="persist", bufs=1))
        x = persist.tile([P, DC, NSEQ], F16)     # resident input (transposed)
        qT = persist.tile([P, NPAIR, NSEQ], F16)  # [hd%128, pair, seq]
        kT = persist.tile([P, NPAIR, NSEQ], F16)
        v = persist.tile([P, NKC, HD], F16)       # [seq%128, seq-chunk, hd]
        wq = persist.tile([P, DC, HD], F16)
        wk = persist.tile([P, DC, HD], F16)
        wv = persist.tile([P, DC, HD], F16)
        wo = persist.tile([P, NPAIR, DMODEL], F16)
        ones = persist.tile([P, 512], F16)
        ones1 = persist.tile([P, 1], F16)
        sel4 = persist.tile([97, P], F16)
        bqs = persist.tile([P, NPAIR], F32)
        bks = persist.tile([P, NPAIR], F32)
        bvr = persist.tile([P, HD], F32)

        nc.sync.dma_start(out=ones, in_=ones_d)
        nc.sync.dma_start(out=ones1, in_=ones1_d)
        nc.sync.dma_start(out=sel4, in_=sel4_d)
        nc.sync.dma_start(out=bqs, in_=bq_d)
        nc.sync.dma_start(out=bks, in_=bk_d)
        nc.sync.dma_start(out=bvr, in_=bvr_d)
        nc.sync.dma_start(out=wv, in_=wv_d.rearrange("(dc p) m -> p dc m", p=P))
        # stage x per seq-chunk so v-projection can start early
        for dc in range(DC):
            nc.sync.dma_start(out=x[:, dc, 0:512], in_=xT_r[:, dc, 0:512])
        nc.sync.dma_start(out=wq, in_=wq_d.rearrange("(dc p) m -> p dc m", p=P))
        nc.sync.dma_start(out=wk, in_=wk_d.rearrange("(dc p) m -> p dc m", p=P))
        for n in range(1, NQC):
            for dc in range(DC):
                nc.sync.dma_start(out=x[:, dc, n * 512:(n + 1) * 512],
                                  in_=xT_r[:, dc, n * 512:(n + 1) * 512])
        nc.sync.dma_start(out=wo, in_=wo_d)

        ssp = ctx.enter_context(tc.tile_pool(name="ssp", bufs=2, space="PSUM"))
        pcp = ctx.enter_context(tc.tile_pool(name="pcp", bufs=2, space="PSUM"))
        dnp = ctx.enter_context(tc.tile_pool(name="dnp", bufs=2, space="PSUM"))
        mkpool = ctx.enter_context(tc.tile_pool(name="mk", bufs=3))
        expool = ctx.enter_context(tc.tile_pool(name="ex", bufs=4))
        exmpool = ctx.enter_context(tc.tile_pool(name="exm", bufs=5))
        cxpool = ctx.enter_context(tc.tile_pool(name="cx", bufs=4))
        trpool = ctx.enter_context(tc.tile_pool(name="tr", bufs=2))
        ypool = ctx.enter_context(tc.tile_pool(name="yo", bufs=3))

        # ---- PE warmup (HAM) ----
        wt = ssp.tile([P, 2, 512], F32, tag="ss", name="warm")
        for i in range(16):
            nc.tensor.matmul(wt[:, i % 2, :], lhsT=ones[:, 0:P], rhs=ones,
                             start=(i < 2), stop=(i >= 14))

        # ---- v projection (all pairs at once, [seq, hd] layout) ----
        for n in range(NQC):
            for s2 in range(2):
                psv = ssp.tile([P, 2, 512], F32, tag="ss", name="psv")
                for j in range(2):
                    s = n * 4 + s2 * 2 + j
                    for dc in range(DC):
                        nc.tensor.matmul(
                            psv[:, j, :],
                            lhsT=x[:, dc, s * 128:(s + 1) * 128],
                            rhs=wv[:, dc, :],
                            start=(dc == 0), stop=(dc == DC - 1))
                for j in range(2):
                    s = n * 4 + s2 * 2 + j
                    nc.vector.tensor_tensor(v[:, s, :], psv[:, j, :], bvr,
                                            OP.add)

        # ---- q/k projection piece: one (q-or-k, seq-chunk) column ----
        def qk_piece(p, idx):
            w_sb, b_sb, dst = ((wq, bqs, qT), (wk, bks, kT))[idx // 4]
            n = idx % 4
            ps = ssp.tile([P, 2, 512], F32, tag="ss", name="qk")
            for dc in range(DC):
                nc.tensor.matmul(
                    ps[:, 0, :],
                    lhsT=w_sb[:, dc, p * 128:(p + 1) * 128],
                    rhs=x[:, dc, n * 512:(n + 1) * 512],
                    start=(dc == 0), stop=(dc == DC - 1))
            nc.scalar.activation(
                out=dst[:, p, n * 512:(n + 1) * 512], in_=ps[:, 0, :],
                func=AF.Identity, bias=b_sb[:, p:p + 1], scale=1.0)

        def qk_proj(p):
            for idx in range(8):
                qk_piece(p, idx)

        qk_proj(0)
        if _NO_QKINT:
            for p in range(1, NPAIR):
                qk_proj(p)

        # ---- output projection for one q-chunk quarter (qs) ----
        def outproj(qc, cps, qs):
            q0 = qc * 512
            py = ssp.tile([P, 2, 512], F32, tag="ss", name="py")
            for dm in range(2):
                for c in range(NPAIR):
                    nc.tensor.matmul(
                        py[:, dm, :],
                        lhsT=cps[c // 2][:, c % 2, qs * 128:(qs + 1) * 128],
                        rhs=wo[:, c, dm * 512:(dm + 1) * 512],
                        start=(c == 0), stop=(c == NPAIR - 1))
            ysb = ypool.tile([P, 2, 512], F32, tag="y")
            nc.scalar.activation(out=ysb, in_=py, func=AF.Copy,
                                 scale=1.0 / 16.0)
            nc.sync.dma_start(
                out=y_d[q0 + qs * 128:q0 + (qs + 1) * 128, :].rearrange(
                    "q (dm n) -> q dm n", n=512),
                in_=ysb)

        # ---- attention ----
        prev = None   # (qc, cps) awaiting deferred output projection
        for qc in range(NQC):
            q0 = qc * 512
            mk_tiles = []
            for j in range(2):
                mk = mkpool.tile([P, 8, 512], F16, tag="mk")
                nc.sync.dma_start(
                    out=mk, in_=maskT_r[:, 8 * j:8 * j + 8, q0:q0 + 512])
                mk_tiles.append(mk)
            cps = []
            for p in range(NPAIR):
                dn = dnp.tile([P, 512], F32, tag="dn", name=f"dn{qc}_{p}")
                if qc == 0 and p < 2:
                    # first use of each dn ring slot: clear power-on garbage
                    # so ln() on rows 1..31 stays finite (1.0 -> ln=0)
                    nc.vector.memset(dn, 1.0)
                pc = pcp.tile([P, 512], F32, tag="pc", name=f"pc{qc}_{p}")
                exm = ex = None
                for kc in range(NKC):
                    ss = ssp.tile([P, 2, 512], F32, tag="ss", name="ss")
                    for hp in range(2):
                        nc.tensor.matmul(
                            ss[:, hp, :],
                            lhsT=kT[64 * hp:64 * hp + 64, p,
                                    kc * 128:(kc + 1) * 128],
                            rhs=qT[64 * hp:64 * hp + 64, p, q0:q0 + 512],
                            start=True, stop=True)
                    if kc % 2 == 0:
                        exm = exmpool.tile([P, 2, 2, 512], F16, tag="exm")
                    mk = mk_tiles[kc // 8]
                    mkb = mk[:, kc % 8, :].unsqueeze(1).broadcast_to(
                        (P, 2, 512))
                    nc.vector.tensor_tensor(exm[:, kc % 2, :, :], ss, mkb,
                                            OP.mult)
                    if kc % 2 == 1:
                        ex = expool.tile([P, 2, 2, 512], F16, tag="ex")
                        nc.scalar.activation(out=ex, in_=exm, func=AF.Exp,
                                             scale=0.125)
                        for j in range(2):
                            kcj = kc - 1 + j
                            for hp in range(2):
                                nc.tensor.matmul(
                                    pc[64 * hp:64 * hp + 64, :],
                                    lhsT=v[:, kcj,
                                           p * 128 + 64 * hp:
                                           p * 128 + 64 * hp + 64],
                                    rhs=ex[:, j, hp, :],
                                    start=(kcj == 0), stop=(kcj == NKC - 1),
                                    tile_position=(0, 64 * hp))
                        if not _NO_DENOM:
                            # 4-slot partial denominators: one col-packed
                            # pass per 2 kc (slots 0/32/64/96)
                            for j in range(2):
                                for hp in range(2):
                                    s = 32 * (2 * j + hp)
                                    nc.tensor.matmul(
                                        dn[s:s + 1, :],
                                        lhsT=ones1[:, 0:1],
                                        rhs=ex[:, j, hp, :],
                                        start=(kc == 1),
                                        stop=(kc == NKC - 1),
                                        tile_position=(0, s))
                    # interleave next pair's q/k projection into qc0 rows
                    if (qc == 0 and p < 3 and not _NO_QKINT
                            and kc % 2 == 1):
                        qk_piece(p + 1, (kc - 1) // 2)
                    # interleave previous q-chunk's output projection into
                    # this q-chunk's first row (keeps DVE fed at boundaries)
                    if p == 0 and prev is not None and kc % 4 == 2:
                        outproj(prev[0], prev[1], kc // 4)
                        if kc == 14:
                            prev = None
                # normalize this pair (two pairs share one fp8 cx tile)
                if p % 2 == 0:
                    cp2 = cxpool.tile([P, 2, 512], F16, tag="cx")
                    cps.append(cp2)
                if _NO_DENOM or _NO_NORM:
                    nc.vector.tensor_copy(out=cp2[:, p % 2, :], in_=pc)
                else:
                    # sum partial denominators + broadcast per head in one
                    # matmul, then 1/d via ln/exp on the broadcast tile
                    dns = trpool.tile([97, 512], F16, tag="dns")
                    nc.scalar.activation(out=dns, in_=dn[0:97, :],
                                         func=AF.Copy, scale=1.0 / 16.0)
                    dsum = dnp.tile([P, 512], F32, tag="dn",
                                    name=f"ds{qc}_{p}")
                    nc.tensor.matmul(dsum, lhsT=sel4, rhs=dns,
                                     start=True, stop=True)
                    t2 = trpool.tile([P, 512], F32, tag="t2")
                    r2 = trpool.tile([P, 512], F32, tag="r2")
                    nc.scalar.activation(out=t2, in_=dsum, func=AF.Ln)
                    nc.scalar.activation(out=r2, in_=t2, func=AF.Exp,
                                         scale=-1.0)
                    nc.vector.tensor_tensor(cp2[:, p % 2, :], pc, r2, OP.mult)
            prev = (qc, cps)
        # final q chunk's output projection (nothing left to overlap)
        for qs in range(4):
            outproj(prev[0], prev[1], qs)
    nc.compile()
    return nc


def _get_nc():
    if "nc" not in _CACHE:
        _CACHE["nc"] = _build()
    return _CACHE["nc"]


def kernel(input, mask, Wq, bq, Wk, bk, Wv, bv, Wo, bo):
    x = np.asarray(input, dtype=np.float32)
    m = np.asarray(mask, dtype=np.float32)
    Wq = np.asarray(Wq, dtype=np.float32)
    Wk = np.asarray(Wk, dtype=np.float32)
    Wv = np.asarray(Wv, dtype=np.float32)
    Wo = np.asarray(Wo, dtype=np.float32)
    bq = np.asarray(bq, dtype=np.float32)
    bk = np.asarray(bk, dtype=np.float32)
    bv = np.asarray(bv, dtype=np.float32)
    bo = np.asarray(bo, dtype=np.float32)
    B = x.shape[0]
    assert x.shape == (B, NSEQ, DMODEL) and B == 4

    sel4 = np.zeros((97, P), np.float16)
    sel4[0, 0:64] = 1.0
    sel4[64, 0:64] = 1.0
    sel4[32, 64:128] = 1.0
    sel4[96, 64:128] = 1.0
    f8 = mybir.dt.np(F8)

    nc = _get_nc()
    in_maps = []
    for b in range(B):
        xT = np.ascontiguousarray(x[b].T)
        mT = np.ascontiguousarray(m[b].T)
        for hg in range(2):
            sl = slice(hg * HD, (hg + 1) * HD)
            in_maps.append({
                "xT": xT.astype(np.float16),
                "maskT": mT.astype(np.float16),
                "wq": np.ascontiguousarray(Wq[:, sl]).astype(np.float16),
                "wk": np.ascontiguousarray(Wk[:, sl]).astype(np.float16),
                "wv": np.ascontiguousarray(Wv[:, sl]).astype(np.float16),
                "wo": np.ascontiguousarray(
                    Wo[sl].reshape(NPAIR, P, DMODEL).transpose(1, 0, 2)
                ).astype(np.float16),
                "bq2": np.ascontiguousarray(bq[sl].reshape(NPAIR, P).T),
                "bk2": np.ascontiguousarray(bk[sl].reshape(NPAIR, P).T),
                "bvr": np.ascontiguousarray(
                    np.broadcast_to(bv[sl], (P, HD))),
                "ones2": np.ones((P, 512), dtype=np.float16),
                "ones1": np.ones((P, 1), dtype=np.float16),
                "sel4": sel4,
            })

    res = bass_utils.run_bass_kernel_spmd(nc, in_maps, core_ids=list(range(8)))
    global LAST_RESULTS
    LAST_RESULTS = res

    out = np.empty((B, NSEQ, DMODEL), dtype=np.float32)
    for b in range(B):
        out[b] = res.results[2 * b]["y"] + res.results[2 * b + 1]["y"] + bo
    return out

